# revision 19
# baseline (speedup 1.0000x reference)
"""Trainium2 Bass kernel for a dense transformer block (RMSNorm + causal MHA +
RMSNorm + SwiGLU MLP, residual), sharded over 8 NeuronCores.

Sharding: core c handles batch b = c//2 and the interleaved 128-row query
blocks g = 2j + (c%2), j in [0,8), of that batch (balances causal attention
work).  All cores run the same SPMD program; per-core differences (owned rows,
causal masks) are carried in the input data.

Math pipeline per core (bf16 data, fp32 accumulation):
  xT = dma-transpose(rmsnorm(x))
  K^T = w_k.T @ xT   Q^T = w_q.T @ xqT            (transposed layouts)
  V   = xT.T @ w_v   (+ ones column -> V_aug)
  S^T = K @ Q^T / 8 -> exp -> mask                (per head, causal-padded)
  [attn^T | rowsum] = V_aug.T @ P^T               (transposed, per q-span)
  attn^T *= bcast(1/rowsum)                       -> attnT tiles directly
  h2 = x + attnT.T @ w_o
  mlp^T = silu(w_gate.T @ hnT) * (w_up.T @ hnT)
  out = h2 + mlp^T.T @ w_down
"""

import numpy as np
import ml_dtypes
import orjson

import bass_rust
import concourse.bass as bass
import concourse.mybir as mybir
from concourse import tile
from concourse.bass_utils import run_bass_kernel_spmd
from concourse.masks import make_identity
from concourse.vector_clock import ScopedClock

BF16 = mybir.dt.bfloat16
F32 = mybir.dt.float32
AF = mybir.ActivationFunctionType

P = 128          # partition dim
D = 1024         # model dim
T = 2048         # sequence length
NH = 16          # heads
DH = 64          # head dim
FF = 4096        # mlp hidden
NB = 8           # owned 128-row q blocks per core
NT = T // P      # 16 t-tiles
ND = D // P      # 8 d-tiles
NF = FF // P     # 32 f-tiles
EPS = 1e-6
N_CORES = 8


class SplitDrainTileContext(tile.TileContext):
    """Workaround: this walrus build accepts only one semaphore wait per
    instruction; the Tile kernel-tail drain accumulates one wait per live
    processor.  Split the waits across individual SP nops."""

    def _drain_and_barrier(self, tick_clock, wait_clock):
        nc = self.nc
        carrier = nc.sync.nop(nofuse=True, hint="tail_wait_carrier")
        wait_clock.add_sem_waits(
            carrier.ins, ScopedClock({None: tick_clock.global_clock})
        )
        si = carrier.ins.sync_info
        waits = list(si.on_wait) if si is not None and si.on_wait else []
        if len(waits) > 1:
            si.on_wait = waits[:1]
            for w in waits[1:]:
                extra = nc.sync.nop(nofuse=True, hint="tail_wait_split")
                if extra.ins.sync_info is None:
                    extra.ins.sync_info = bass_rust.SyncInfo(
                        on_wait=[w], on_update=[]
                    )
                else:
                    extra.ins.sync_info.on_wait = [w]
        nc.sync.drain()
        nc.all_engine_barrier()
        assert self.sems is not None
        popped = nc._tile_sem_poison_stack.pop()
        assert popped is self._sem_poison
        nc.clear_and_free_semaphores(list(self.sems.allocated().values()))
        nc.all_engine_barrier()


def _split_multi_waits(bir: dict) -> dict:
    """This walrus build accepts only one semaphore wait per instruction;
    Tile attaches several.  Split the extras into standalone EventSemaphore
    (wait-only) instructions inserted immediately before the offender in the
    same block, preserving per-engine program order."""
    ctr = [0]

    def fix_block(block):
        insts = block.get("instructions")
        if insts:
            fixed = []
            for inst in insts:
                si = inst.get("sync_info")
                waits = (si or {}).get("on_wait") or []
                if len(waits) > 1:
                    for w in waits[:-1]:
                        ctr[0] += 1
                        fixed.append({
                            "debug": inst.get("debug", 0),
                            "engine": inst["engine"],
                            "ins": [],
                            "outs": [],
                            "name": f"evw_{ctr[0]}",
                            "opcode": "EventSemaphore",
                            "sync_info": {"on_update": [], "on_wait": [w]},
                        })
                    si["on_wait"] = [waits[-1]]
                fixed.append(inst)
            block["instructions"] = fixed
        for sub in block.get("blocks") or []:
            fix_block(sub)

    for fn in bir["functions"]:
        for b in fn.get("blocks") or []:
            fix_block(b)
    return bir


def _dedupe_ldweights(bir: dict) -> dict:
    """Drop an Ldweights whose stationary operand is identical to the
    previous Ldweights in the same PE stream with only Matmult /
    EventSemaphore PE instructions in between: the weights are already in
    the array.  Waits/updates of a dropped instruction are preserved on a
    standalone EventSemaphore.  (This walrus build serializes each
    LDWEIGHTS with its MATMUL, so fewer loads = directly less PE time.)"""
    ctr = [0]

    def sig_of(inst):
        return orjson.dumps(
            [inst.get("ins"), inst.get("tile_position"), inst.get("tile_size")]
        )

    def fix_block(block):
        insts = block.get("instructions")
        if insts:
            fixed = []
            last_sig = None
            for inst in insts:
                if inst.get("engine") == "PE":
                    op = inst.get("opcode")
                    if op == "Ldweights":
                        s = sig_of(inst)
                        if s == last_sig:
                            si = inst.get("sync_info") or {}
                            if si.get("on_wait") or si.get("on_update"):
                                ctr[0] += 1
                                fixed.append({
                                    "debug": inst.get("debug", 0),
                                    "engine": "PE",
                                    "ins": [],
                                    "outs": [],
                                    "name": f"ldwev_{ctr[0]}",
                                    "opcode": "EventSemaphore",
                                    "sync_info": si,
                                })
                            continue
                        last_sig = s
                    elif op in ("Matmult", "EventSemaphore", "NoOp"):
                        pass
                    else:
                        last_sig = None
                fixed.append(inst)
            block["instructions"] = fixed
        for sub in block.get("blocks") or []:
            fix_block(sub)

    for fn in bir["functions"]:
        for b in fn.get("blocks") or []:
            fix_block(b)
    return bir


def build_nc(sim_compat=False):
    nc = bass.Bass("TRN2")

    xbt = nc.dram_tensor("xbt", [T, D], BF16, kind="ExternalInput")
    xq = nc.dram_tensor("xq", [NB * P, D], BF16, kind="ExternalInput")
    xres = nc.dram_tensor("xres", [NB * P, D], F32, kind="ExternalInput")
    wqkv = nc.dram_tensor("wqkv", [D, 3 * D], BF16, kind="ExternalInput")
    wo = nc.dram_tensor("wo", [D, D], BF16, kind="ExternalInput")
    wup = nc.dram_tensor("wup", [D, FF], BF16, kind="ExternalInput")
    wgate = nc.dram_tensor("wgate", [D, FF], BF16, kind="ExternalInput")
    wdown = nc.dram_tensor("wdown", [FF, D], BF16, kind="ExternalInput")
    maskt = nc.dram_tensor("maskt", [NB, 2, P, P], BF16, kind="ExternalInput")
    out = nc.dram_tensor("out", [NB * P, D], F32, kind="ExternalOutput")

    with SplitDrainTileContext(nc) as tc:
        with (
            tc.tile_pool(name="const", bufs=1) as constp,
            tc.tile_pool(name="ps", bufs=4, space="PSUM") as psp,
            tc.tile_pool(name="stats", bufs=1) as statsp,
            tc.tile_pool(name="io", bufs=4) as iop,
            tc.tile_pool(name="work", bufs=4) as workp,
            tc.tile_pool(name="attnT", bufs=1) as attnTp,
        ):
            ident = constp.tile([P, P], BF16)
            make_identity(nc, ident[:])
            eps_t = constp.tile([P, 1], F32)
            nc.gpsimd.memset(eps_t[:], EPS)
            ones_row = constp.tile([1, DH], BF16)
            nc.gpsimd.memset(ones_row[:], 1.0)
            mask_sb = constp.tile([P, NB * 2 * P], BF16)
            for j in range(NB):
                for i in range(2):
                    col = (j * 2 + i) * P
                    nc.sync.dma_start(
                        mask_sb[:, col : col + P], maskt[j, i, :, :]
                    )
            mask_v = mask_sb[:].rearrange("p (a b q) -> p a b q", a=NB, b=2)

            # attn^T tiles, pair-major: tile p holds heads 2p (partitions
            # 0:64) and 2p+1 (64:128) over the owned 1024 q columns --
            # exactly the dm-tile layout the O projection needs.
            attnT = [
                attnTp.tile([P, NB * P], BF16, tag=f"aT{j}", name=f"aT{j}")
                for j in range(ND)
            ]

            def transpose_128(dst_ap, src_ap):
                """dst[128, 128] = src[128, 128].T via PE transpose + DVE
                evict.  (DMA X-bar transpose serializes against every other
                DMA through the xbar-mode switch -- measured ~1.3us each.)"""
                pst = psp.tile([P, P], BF16, tag="ps", name="pst")
                nc.tensor.transpose(pst[:], src_ap, ident[:])
                nc.vector.tensor_copy(dst_ap, pst[:])

            with (
                tc.tile_pool(name="xT", bufs=1) as xTp,
                tc.tile_pool(name="xqT", bufs=1) as xqTp,
            ):
                xT = [
                    xTp.tile([P, T], BF16, tag=f"xT{j}", name=f"xT{j}")
                    for j in range(ND)
                ]
                xqT = [
                    xqTp.tile([P, NB * P], BF16, tag=f"xqT{j}", name=f"xqT{j}")
                    for j in range(ND)
                ]

                # ---- Stage A: stats + normalize + transpose (x and xq)
                with tc.tile_pool(name="xnat", bufs=1) as xnatp:
                    sumsq = statsp.tile([P, NT], F32)
                    rms = statsp.tile([P, NT], F32)
                    invr = statsp.tile([P, NT], F32)
                    sumsq_q = statsp.tile([P, NB], F32)
                    rms_q = statsp.tile([P, NB], F32)
                    invr_q = statsp.tile([P, NB], F32)
                    xnat = []
                    xqnat = []
                    for i in range(NT):
                        xt = xnatp.tile([P, D], BF16, tag=f"xn{i}")
                        nc.sync.dma_start(xt[:], xbt[i * P : (i + 1) * P, :])
                        scr = workp.tile([P, D], BF16, tag="sq_scr")
                        nc.scalar.activation(
                            scr[:], xt[:], AF.Square,
                            accum_out=sumsq[:, i : i + 1],
                        )
                        xnat.append(xt)
                    for i in range(NB):
                        xt = xnatp.tile([P, D], BF16, tag=f"xqn{i}")
                        nc.sync.dma_start(xt[:], xq[i * P : (i + 1) * P, :])
                        scr = workp.tile([P, D], BF16, tag="sq_scr")
                        nc.scalar.activation(
                            scr[:], xt[:], AF.Square,
                            accum_out=sumsq_q[:, i : i + 1],
                        )
                        xqnat.append(xt)

                    # per-tile sqrt/recip so normalize+transpose of tile i
                    # doesn't wait for all tiles' stats
                    for i in range(NT):
                        nc.scalar.activation(
                            rms[:, i : i + 1], sumsq[:, i : i + 1],
                            AF.Sqrt, bias=eps_t[:], scale=1.0 / D,
                        )
                        nc.vector.reciprocal(
                            invr[:, i : i + 1], rms[:, i : i + 1]
                        )
                        xn = workp.tile([P, D], BF16, tag="xn")
                        nc.scalar.activation(
                            xn[:], xnat[i][:], AF.Copy,
                            scale=invr[:, i : i + 1],
                        )
                        for j in range(ND):
                            transpose_128(
                                xT[j][:, i * P : (i + 1) * P],
                                xn[:, j * P : (j + 1) * P],
                            )
                    for i in range(NB):
                        nc.scalar.activation(
                            rms_q[:, i : i + 1], sumsq_q[:, i : i + 1],
                            AF.Sqrt, bias=eps_t[:], scale=1.0 / D,
                        )
                        nc.vector.reciprocal(
                            invr_q[:, i : i + 1], rms_q[:, i : i + 1]
                        )
                        xn = workp.tile([P, D], BF16, tag="xn")
                        nc.scalar.activation(
                            xn[:], xqnat[i][:], AF.Copy,
                            scale=invr_q[:, i : i + 1],
                        )
                        for j in range(ND):
                            transpose_128(
                                xqT[j][:, i * P : (i + 1) * P],
                                xn[:, j * P : (j + 1) * P],
                            )

                # ---- Stage B: V (natural layout) with ones column appended
                with tc.tile_pool(name="vaug", bufs=1) as vaugp:
                    vaug = [
                        vaugp.tile(
                            [P, NH * (DH + 1)], BF16,
                            tag=f"va{t}", name=f"va{t}",
                        )
                        for t in range(NT)
                    ]
                    with tc.tile_pool(name="wv", bufs=1) as wvp:
                        wv_t = [
                            wvp.tile([P, D], BF16, tag=f"wv{j}", name=f"wv{j}")
                            for j in range(ND)
                        ]
                        for j in range(ND):
                            nc.sync.dma_start(
                                wv_t[j][:],
                                wqkv[j * P : (j + 1) * P, 2 * D : 3 * D],
                            )
                        for t in range(NT):
                            ones_view = vaug[t][:].rearrange(
                                "p (h c) -> p h c", c=DH + 1
                            )[:, :, DH : DH + 1]
                            nc.gpsimd.memset(ones_view, 1.0)
                            psv = [
                                psp.tile(
                                    [P, 512], F32, tag="ps", name=f"psv{cs}"
                                )
                                for cs in range(2)
                            ]
                            # j outer so both c-spans share one LDWEIGHTS
                            for j in range(ND):
                                for cs in range(2):
                                    nc.tensor.matmul(
                                        psv[cs][:],
                                        lhsT=xT[j][:, t * P : (t + 1) * P],
                                        rhs=wv_t[j][
                                            :, cs * 512 : (cs + 1) * 512
                                        ],
                                        start=(j == 0),
                                        stop=(j == ND - 1),
                                    )
                            for cs in range(2):
                                dst = vaug[t][:].rearrange(
                                    "p (h c) -> p h c", c=DH + 1
                                )[:, cs * 8 : (cs + 1) * 8, 0:DH]
                                src = psv[cs][:].rearrange(
                                    "p (a c) -> p a c", c=DH
                                )
                                nc.vector.tensor_copy(dst, src)

                    # ---- Stage C: per head pair: K^T/Q^T proj + attention
                    with (
                        tc.tile_pool(name="wqk", bufs=1) as wqkp,
                        tc.tile_pool(name="ktqt", bufs=2) as ktqtp,
                        tc.tile_pool(name="pt", bufs=8) as ptp,
                    ):
                        wq_t = [
                            wqkp.tile([P, D], BF16, tag=f"wq{j}", name=f"wq{j}")
                            for j in range(ND)
                        ]
                        wk_t = [
                            wqkp.tile([P, D], BF16, tag=f"wk{j}", name=f"wk{j}")
                            for j in range(ND)
                        ]
                        for j in range(ND):
                            nc.sync.dma_start(
                                wq_t[j][:], wqkv[j * P : (j + 1) * P, 0:D]
                            )
                            nc.sync.dma_start(
                                wk_t[j][:], wqkv[j * P : (j + 1) * P, D : 2 * D]
                            )

                        for p in range(NH // 2):
                            KT = ktqtp.tile([P, T], BF16, tag="KT")
                            QT = ktqtp.tile([P, NB * P], BF16, tag="QT")
                            # j outer in groups of 2 t-spans: one LDWEIGHTS
                            # per 2 matmuls while holding only 2 psum banks
                            # (4 banks starved the concurrent attention spans)
                            for tg in range(2):
                                psk = [
                                    psp.tile(
                                        [P, 512], F32, tag="ps",
                                        name=f"psk{ts}",
                                    )
                                    for ts in range(2)
                                ]
                                for j in range(ND):
                                    for ts in range(2):
                                        t4 = tg * 2 + ts
                                        nc.tensor.matmul(
                                            psk[ts][:],
                                            lhsT=wk_t[j][
                                                :, p * P : (p + 1) * P
                                            ],
                                            rhs=xT[j][
                                                :, t4 * 512 : (t4 + 1) * 512
                                            ],
                                            start=(j == 0),
                                            stop=(j == ND - 1),
                                        )
                                for ts in range(2):
                                    t4 = tg * 2 + ts
                                    nc.vector.tensor_copy(
                                        KT[:, t4 * 512 : (t4 + 1) * 512],
                                        psk[ts][:],
                                    )
                            psq = [
                                psp.tile(
                                    [P, 512], F32, tag="ps", name=f"psq{ts}"
                                )
                                for ts in range(2)
                            ]
                            for j in range(ND):
                                for ts in range(2):
                                    nc.tensor.matmul(
                                        psq[ts][:],
                                        lhsT=wq_t[j][:, p * P : (p + 1) * P],
                                        rhs=xqT[j][
                                            :, ts * 512 : (ts + 1) * 512
                                        ],
                                        start=(j == 0),
                                        stop=(j == ND - 1),
                                    )
                            for ts in range(2):
                                nc.vector.tensor_copy(
                                    QT[:, ts * 512 : (ts + 1) * 512],
                                    psq[ts][:],
                                )

                            # attention over q-spans of 256 (owned blocks
                            # 2s, 2s+1); k extent padded to 4s+4 tiles so
                            # the loop structure is h-independent.
                            for s in range(4):
                                nk = 4 * s + 4
                                psavT = {}
                                for hl in range(2):
                                    psavT[hl] = psp.tile(
                                        [P, 256], F32, tag="ps",
                                        name=f"psavT{hl}",
                                    )
                                    psavT[hl] = psavT[hl][:, :]
                                # 4 k-tiles share one psum pair + one exp
                                for kt4 in range(nk // 4):
                                    for hl in range(2):
                                        pss = psp.tile(
                                            [P, 1024], F32, tag="ps2",
                                            bufs=2,
                                        )
                                        for ktl in range(4):
                                            kt = 4 * kt4 + ktl
                                            nc.tensor.matmul(
                                                pss[
                                                    :,
                                                    ktl * 256 : (ktl + 1) * 256,
                                                ],
                                                lhsT=KT[
                                                    hl * DH : (hl + 1) * DH,
                                                    kt * P : (kt + 1) * P,
                                                ],
                                                rhs=QT[
                                                    hl * DH : (hl + 1) * DH,
                                                    s * 256 : (s + 1) * 256,
                                                ],
                                                start=True,
                                                stop=True,
                                            )
                                        pt = ptp.tile([P, 1024], BF16, tag="pt")
                                        nc.scalar.activation(
                                            pt[:], pss[:], AF.Exp, scale=0.125
                                        )
                                        hh = 2 * p + hl
                                        for ktl in range(4):
                                            kt = 4 * kt4 + ktl
                                            ko = ktl * 256
                                            # diagonal-block masks
                                            for bl in range(2):
                                                b = 2 * s + bl
                                                if kt in (2 * b, 2 * b + 1):
                                                    i = kt - 2 * b
                                                    sl = pt[
                                                        :,
                                                        ko + bl * P : ko
                                                        + (bl + 1) * P,
                                                    ]
                                                    nc.vector.tensor_mul(
                                                        sl,
                                                        sl,
                                                        mask_v[:, b, i, :],
                                                    )
                                            # block 2s is done after
                                            # 4s+2 k-tiles; zero its pt
                                            # columns on the pad tiles
                                            if kt >= 4 * s + 2:
                                                nc.vector.memset(
                                                    pt[:, ko : ko + P], 0.0
                                                )
                                            # transposed AV + rowsum:
                                            # psavT[0:64] = attn^T,
                                            # psavT[64] = softmax denom
                                            nc.tensor.matmul(
                                                psavT[hl][0 : DH + 1, :],
                                                lhsT=vaug[kt][
                                                    :,
                                                    hh * (DH + 1) : (hh + 1)
                                                    * (DH + 1),
                                                ],
                                                rhs=pt[:, ko : ko + 256],
                                                start=(kt == 0),
                                                stop=(kt == nk - 1),
                                                skip_group_check=True,
                                            )
                                # broadcast both rowsum rows across the 64
                                # head dims via K=1 matmuls with a ones
                                # column, then one reciprocal on [64, 512]
                                # (a [1, 256] reciprocal runs on a single
                                # DVE lane: ~1.2us)
                                rsum = workp.tile([1, 512], BF16, tag="rsum")
                                bc_ps = psp.tile(
                                    [DH, 512], F32, tag="ps", name="bc_ps"
                                )
                                for hl in range(2):
                                    nc.vector.tensor_copy(
                                        rsum[:, hl * 256 : (hl + 1) * 256],
                                        psavT[hl][DH : DH + 1, :],
                                    )
                                    nc.tensor.matmul(
                                        bc_ps[:, hl * 256 : (hl + 1) * 256],
                                        lhsT=ones_row[:],
                                        rhs=rsum[:, hl * 256 : (hl + 1) * 256],
                                        start=True,
                                        stop=True,
                                        skip_group_check=True,
                                    )
                                bc = workp.tile([DH, 512], F32, tag="bc")
                                nc.vector.reciprocal(bc[:], bc_ps[:])
                                for hl in range(2):
                                    nc.vector.tensor_mul(
                                        attnT[p][
                                            hl * DH : (hl + 1) * DH,
                                            s * 256 : (s + 1) * 256,
                                        ],
                                        psavT[hl][0:DH, :],
                                        bc[:, hl * 256 : (hl + 1) * 256],
                                    )

            # ---- Stage E: O proj + residual, rmsnorm2
            with tc.tile_pool(name="h2", bufs=1) as h2p:
                h2 = [
                    h2p.tile([P, D], F32, tag=f"h2{b}", name=f"h2{b}")
                    for b in range(NB)
                ]
                with (
                    tc.tile_pool(name="wo", bufs=1) as wop,
                    tc.tile_pool(name="io2", bufs=4) as io2p,
                ):
                    wo_t = [
                        wop.tile([P, D], BF16, tag=f"wo{j}", name=f"wot{j}")
                        for j in range(ND)
                    ]
                    for j in range(ND):
                        nc.sync.dma_start(
                            wo_t[j][:], wo[j * P : (j + 1) * P, :]
                        )
                    sumsq2 = statsp.tile([P, NB], F32)
                    for b in range(NB):
                        xr = io2p.tile([P, D], F32, tag="xres")
                        nc.sync.dma_start(
                            xr[:], xres[b * P : (b + 1) * P, :]
                        )
                        pso = [
                            psp.tile([P, 512], F32, tag="ps", name=f"pso{es}")
                            for es in range(2)
                        ]
                        # j outer: both e-spans share one LDWEIGHTS
                        for j in range(ND):
                            for es in range(2):
                                nc.tensor.matmul(
                                    pso[es][:],
                                    lhsT=attnT[j][:, b * P : (b + 1) * P],
                                    rhs=wo_t[j][:, es * 512 : (es + 1) * 512],
                                    start=(j == 0),
                                    stop=(j == ND - 1),
                                )
                        for es in range(2):
                            nc.vector.tensor_add(
                                h2[b][:, es * 512 : (es + 1) * 512],
                                pso[es][:],
                                xr[:, es * 512 : (es + 1) * 512],
                            )
                        scr = workp.tile([P, D], BF16, tag="sq_scr")
                        nc.scalar.activation(
                            scr[:], h2[b][:], AF.Square,
                            accum_out=sumsq2[:, b : b + 1],
                        )

                # ---- Stage F: MLP (transposed up/gate, natural down)
                with (
                    tc.tile_pool(name="hnT", bufs=1) as hnTp,
                    tc.tile_pool(name="mlpT", bufs=1) as mlpTp,
                    tc.tile_pool(name="ws", bufs=3) as wsp,
                    tc.tile_pool(name="io3", bufs=4) as io3p,
                ):
                    rms2 = statsp.tile([P, NB], F32)
                    invr2 = statsp.tile([P, NB], F32)

                    hnT = [
                        hnTp.tile(
                            [P, NB * P], BF16, tag=f"hT{j}", name=f"hT{j}"
                        )
                        for j in range(ND)
                    ]
                    for b in range(NB):
                        nc.scalar.activation(
                            rms2[:, b : b + 1], sumsq2[:, b : b + 1],
                            AF.Sqrt, bias=eps_t[:], scale=1.0 / D,
                        )
                        nc.vector.reciprocal(
                            invr2[:, b : b + 1], rms2[:, b : b + 1]
                        )
                        hn = workp.tile([P, D], BF16, tag="xn")
                        nc.scalar.activation(
                            hn[:], h2[b][:], AF.Copy,
                            scale=invr2[:, b : b + 1],
                        )
                        for j in range(ND):
                            transpose_128(
                                hnT[j][:, b * P : (b + 1) * P],
                                hn[:, j * P : (j + 1) * P],
                            )

                    mlpT = [
                        mlpTp.tile(
                            [P, NB * P], BF16, tag=f"m{ft}", name=f"mT{ft}"
                        )
                        for ft in range(NF)
                    ]
                    for ft in range(NF):
                        wu = wsp.tile([P, D], BF16, tag="wu")
                        nc.sync.dma_start(
                            wu[:].rearrange("p (a f) -> p a f", f=P),
                            wup[:, ft * P : (ft + 1) * P].rearrange(
                                "(a p) f -> p a f", p=P
                            ),
                        )
                        wg = wsp.tile([P, D], BF16, tag="wg")
                        nc.sync.dma_start(
                            wg[:].rearrange("p (a f) -> p a f", f=P),
                            wgate[:, ft * P : (ft + 1) * P].rearrange(
                                "(a p) f -> p a f", p=P
                            ),
                        )
                        psg = [
                            psp.tile([P, 512], F32, tag="ps", name=f"psg{qs}")
                            for qs in range(2)
                        ]
                        psu = [
                            psp.tile([P, 512], F32, tag="ps", name=f"psu{qs}")
                            for qs in range(2)
                        ]
                        # j outer: both q-spans share one LDWEIGHTS
                        for j in range(ND):
                            for qs in range(2):
                                nc.tensor.matmul(
                                    psg[qs][:],
                                    lhsT=wg[:, j * P : (j + 1) * P],
                                    rhs=hnT[j][:, qs * 512 : (qs + 1) * 512],
                                    start=(j == 0),
                                    stop=(j == ND - 1),
                                )
                        for j in range(ND):
                            for qs in range(2):
                                nc.tensor.matmul(
                                    psu[qs][:],
                                    lhsT=wu[:, j * P : (j + 1) * P],
                                    rhs=hnT[j][:, qs * 512 : (qs + 1) * 512],
                                    start=(j == 0),
                                    stop=(j == ND - 1),
                                )
                        for qs in range(2):
                            if sim_compat:
                                # CoreSim lacks Silu: silu(g) = g*sigmoid(g)
                                sg = workp.tile([P, 512], BF16, tag="sg")
                                nc.scalar.activation(
                                    sg[:], psg[qs][:], AF.Sigmoid
                                )
                                tmp = workp.tile([P, 512], BF16, tag="sgt")
                                nc.vector.tensor_mul(
                                    tmp[:], psg[qs][:], sg[:]
                                )
                                nc.vector.tensor_mul(
                                    mlpT[ft][:, qs * 512 : (qs + 1) * 512],
                                    psu[qs][:],
                                    tmp[:],
                                )
                            else:
                                sg = workp.tile([P, 512], BF16, tag="sg")
                                nc.scalar.activation(
                                    sg[:], psg[qs][:], AF.Silu
                                )
                                nc.vector.tensor_mul(
                                    mlpT[ft][:, qs * 512 : (qs + 1) * 512],
                                    psu[qs][:],
                                    sg[:],
                                )

                    # down projection + final residual, two q-blocks at a
                    # time; each block's psum accumulator holds both e-spans
                    # (2 banks) and each mlpT LDWEIGHTS serves both
                    for bq in range(4):
                        psd = {}
                        for bi in range(2):
                            psd[bi] = psp.tile(
                                [P, D], F32, tag="ps2", bufs=2,
                                name=f"psd{bi}",
                            )
                        for ft in range(NF):
                            wd = wsp.tile([P, D], BF16, tag="wd")
                            nc.sync.dma_start(
                                wd[:], wdown[ft * P : (ft + 1) * P, :]
                            )
                            for bi in range(2):
                                b = bq * 2 + bi
                                for es in range(2):
                                    nc.tensor.matmul(
                                        psd[bi][
                                            :, es * 512 : (es + 1) * 512
                                        ],
                                        lhsT=mlpT[ft][:, b * P : (b + 1) * P],
                                        rhs=wd[:, es * 512 : (es + 1) * 512],
                                        start=(ft == 0),
                                        stop=(ft == NF - 1),
                                        skip_group_check=True,
                                    )
                        for bi in range(2):
                            b = bq * 2 + bi
                            for es in range(2):
                                ot = io3p.tile([P, 512], F32, tag="outt")
                                nc.vector.tensor_add(
                                    ot[:],
                                    psd[bi][:, es * 512 : (es + 1) * 512],
                                    h2[b][:, es * 512 : (es + 1) * 512],
                                )
                                nc.sync.dma_start(
                                    out[
                                        b * P : (b + 1) * P,
                                        es * 512 : (es + 1) * 512,
                                    ],
                                    ot[:],
                                )

    orig_to_json_bytes = nc.to_json_bytes

    def _patched_to_json_bytes():
        bir = orjson.loads(orig_to_json_bytes())
        bir = _split_multi_waits(bir)
        bir = _dedupe_ldweights(bir)
        return orjson.dumps(bir)

    nc.to_json_bytes = _patched_to_json_bytes
    return nc


_NC_CACHE = {}


def _get_nc(sim_compat=False):
    if sim_compat not in _NC_CACHE:
        _NC_CACHE[sim_compat] = build_nc(sim_compat)
    return _NC_CACHE[sim_compat]


def _prep_core_inputs(x, w_qkv, w_o, w_up, w_gate, w_down, scale1, scale2):
    bf = ml_dtypes.bfloat16
    wqkv_f = (scale1[:, None].astype(np.float64) * w_qkv.astype(np.float64))
    wup_f = (scale2[:, None].astype(np.float64) * w_up.astype(np.float64))
    wgate_f = (scale2[:, None].astype(np.float64) * w_gate.astype(np.float64))
    shared = {
        "wqkv": wqkv_f.astype(bf),
        "wo": w_o.astype(bf),
        "wup": wup_f.astype(bf),
        "wgate": wgate_f.astype(bf),
        "wdown": w_down.astype(bf),
    }
    in_maps = []
    for c in range(N_CORES):
        b, h = divmod(c, 2)
        xb = np.asarray(x[b], dtype=np.float32)
        own = np.concatenate(
            [xb[(2 * j + h) * P : (2 * j + h + 1) * P] for j in range(NB)]
        )
        mask = np.zeros((NB, 2, P, P), dtype=np.float32)
        kl = np.arange(P)[:, None]
        ql = np.arange(P)[None, :]
        for j in range(NB):
            g = 2 * j + h
            for i in range(2):
                kg = (2 * j + i) * P + kl
                qg = g * P + ql
                mask[j, i] = (kg <= qg).astype(np.float32)
        m = dict(shared)
        m["xbt"] = xb.astype(bf)
        m["xq"] = own.astype(bf)
        m["xres"] = own
        m["maskt"] = mask.astype(bf)
        in_maps.append(m)
    return in_maps


def _assemble(results):
    out = np.zeros((4, T, D), dtype=np.float32)
    for c in range(N_CORES):
        b, h = divmod(c, 2)
        o = results[c]["out"]
        for j in range(NB):
            g = 2 * j + h
            out[b, g * P : (g + 1) * P, :] = o[j * P : (j + 1) * P, :]
    return out


def kernel(x, w_qkv, w_o, w_up, w_gate, w_down, scale1, scale2):
    x = np.asarray(x, dtype=np.float32)
    in_maps = _prep_core_inputs(
        x,
        np.asarray(w_qkv, dtype=np.float32),
        np.asarray(w_o, dtype=np.float32),
        np.asarray(w_up, dtype=np.float32),
        np.asarray(w_gate, dtype=np.float32),
        np.asarray(w_down, dtype=np.float32),
        np.asarray(scale1, dtype=np.float32),
        np.asarray(scale2, dtype=np.float32),
    )
    nc = _get_nc()
    res = run_bass_kernel_spmd(nc, in_maps, list(range(N_CORES)))
    return _assemble(res.results)


# revision 20
# speedup vs baseline: 1.3581x; 1.3581x over previous
"""Trainium2 Bass kernel for a dense transformer block (RMSNorm + causal MHA +
RMSNorm + SwiGLU MLP, residual), sharded over 8 NeuronCores.

Sharding: core c handles batch b = c//2 and the interleaved 128-row query
blocks g = 2j + (c%2), j in [0,8), of that batch (balances causal attention
work).  All cores run the same SPMD program; per-core differences (owned rows,
causal masks) are carried in the input data.

Math pipeline per core (bf16 data, fp32 accumulation):
  xT = dma-transpose(rmsnorm(x))
  K^T = w_k.T @ xT   Q^T = w_q.T @ xqT            (transposed layouts)
  V   = xT.T @ w_v   (+ ones column -> V_aug)
  S^T = K @ Q^T / 8 -> exp -> mask                (per head, causal-padded)
  [attn^T | rowsum] = V_aug.T @ P^T               (transposed, per q-span)
  attn^T *= bcast(1/rowsum)                       -> attnT tiles directly
  h2 = x + attnT.T @ w_o
  mlp^T = silu(w_gate.T @ hnT) * (w_up.T @ hnT)
  out = h2 + mlp^T.T @ w_down
"""

import numpy as np
import ml_dtypes
import orjson

import bass_rust
import concourse.bass as bass
import concourse.mybir as mybir
from concourse import tile
from concourse.bass_utils import run_bass_kernel_spmd
from concourse.masks import make_identity
from concourse.vector_clock import ScopedClock

BF16 = mybir.dt.bfloat16
F32 = mybir.dt.float32
AF = mybir.ActivationFunctionType

P = 128          # partition dim
D = 1024         # model dim
T = 2048         # sequence length
NH = 16          # heads
DH = 64          # head dim
FF = 4096        # mlp hidden
NB = 8           # owned 128-row q blocks per core
NT = T // P      # 16 t-tiles
ND = D // P      # 8 d-tiles
NF = FF // P     # 32 f-tiles
EPS = 1e-6
N_CORES = 8


class SplitDrainTileContext(tile.TileContext):
    """Workaround: this walrus build accepts only one semaphore wait per
    instruction; the Tile kernel-tail drain accumulates one wait per live
    processor.  Split the waits across individual SP nops."""

    def _drain_and_barrier(self, tick_clock, wait_clock):
        nc = self.nc
        carrier = nc.sync.nop(nofuse=True, hint="tail_wait_carrier")
        wait_clock.add_sem_waits(
            carrier.ins, ScopedClock({None: tick_clock.global_clock})
        )
        si = carrier.ins.sync_info
        waits = list(si.on_wait) if si is not None and si.on_wait else []
        if len(waits) > 1:
            si.on_wait = waits[:1]
            for w in waits[1:]:
                extra = nc.sync.nop(nofuse=True, hint="tail_wait_split")
                if extra.ins.sync_info is None:
                    extra.ins.sync_info = bass_rust.SyncInfo(
                        on_wait=[w], on_update=[]
                    )
                else:
                    extra.ins.sync_info.on_wait = [w]
        nc.sync.drain()
        nc.all_engine_barrier()
        assert self.sems is not None
        popped = nc._tile_sem_poison_stack.pop()
        assert popped is self._sem_poison
        nc.clear_and_free_semaphores(list(self.sems.allocated().values()))
        nc.all_engine_barrier()


def _split_multi_waits(bir: dict) -> dict:
    """This walrus build accepts only one semaphore wait per instruction;
    Tile attaches several.  Split the extras into standalone EventSemaphore
    (wait-only) instructions inserted immediately before the offender in the
    same block, preserving per-engine program order."""
    ctr = [0]

    def fix_block(block):
        insts = block.get("instructions")
        if insts:
            fixed = []
            for inst in insts:
                si = inst.get("sync_info")
                waits = (si or {}).get("on_wait") or []
                if len(waits) > 1:
                    for w in waits[:-1]:
                        ctr[0] += 1
                        fixed.append({
                            "debug": inst.get("debug", 0),
                            "engine": inst["engine"],
                            "ins": [],
                            "outs": [],
                            "name": f"evw_{ctr[0]}",
                            "opcode": "EventSemaphore",
                            "sync_info": {"on_update": [], "on_wait": [w]},
                        })
                    si["on_wait"] = [waits[-1]]
                fixed.append(inst)
            block["instructions"] = fixed
        for sub in block.get("blocks") or []:
            fix_block(sub)

    for fn in bir["functions"]:
        for b in fn.get("blocks") or []:
            fix_block(b)
    return bir


def _dedupe_ldweights(bir: dict) -> dict:
    """Drop an Ldweights whose stationary operand is identical to the
    previous Ldweights in the same PE stream with only Matmult /
    EventSemaphore PE instructions in between: the weights are already in
    the array.  Waits/updates of a dropped instruction are preserved on a
    standalone EventSemaphore.  (This walrus build serializes each
    LDWEIGHTS with its MATMUL, so fewer loads = directly less PE time.)"""
    ctr = [0]

    def sig_of(inst):
        return orjson.dumps(
            [inst.get("ins"), inst.get("tile_position"), inst.get("tile_size")]
        )

    def fix_block(block):
        insts = block.get("instructions")
        if insts:
            fixed = []
            last_sig = None
            for inst in insts:
                if inst.get("engine") == "PE":
                    op = inst.get("opcode")
                    if op == "Ldweights":
                        s = sig_of(inst)
                        if s == last_sig:
                            si = inst.get("sync_info") or {}
                            if si.get("on_wait") or si.get("on_update"):
                                ctr[0] += 1
                                fixed.append({
                                    "debug": inst.get("debug", 0),
                                    "engine": "PE",
                                    "ins": [],
                                    "outs": [],
                                    "name": f"ldwev_{ctr[0]}",
                                    "opcode": "EventSemaphore",
                                    "sync_info": si,
                                })
                            continue
                        last_sig = s
                    elif op in ("Matmult", "EventSemaphore", "NoOp"):
                        pass
                    else:
                        last_sig = None
                fixed.append(inst)
            block["instructions"] = fixed
        for sub in block.get("blocks") or []:
            fix_block(sub)

    for fn in bir["functions"]:
        for b in fn.get("blocks") or []:
            fix_block(b)
    return bir


def build_nc(sim_compat=False):
    nc = bass.Bass("TRN2")

    xbt = nc.dram_tensor("xbt", [T, D], BF16, kind="ExternalInput")
    xq = nc.dram_tensor("xq", [NB * P, D], BF16, kind="ExternalInput")
    xres = nc.dram_tensor("xres", [NB * P, D], F32, kind="ExternalInput")
    wqkv = nc.dram_tensor("wqkv", [D, 3 * D], BF16, kind="ExternalInput")
    wo = nc.dram_tensor("wo", [D, D], BF16, kind="ExternalInput")
    wup = nc.dram_tensor("wup", [D, FF], BF16, kind="ExternalInput")
    wgate = nc.dram_tensor("wgate", [D, FF], BF16, kind="ExternalInput")
    wdown = nc.dram_tensor("wdown", [FF, D], BF16, kind="ExternalInput")
    maskt = nc.dram_tensor("maskt", [NB, 2, P, P], BF16, kind="ExternalInput")
    out = nc.dram_tensor("out", [NB * P, D], F32, kind="ExternalOutput")

    with SplitDrainTileContext(nc) as tc:
        with (
            tc.tile_pool(name="const", bufs=1) as constp,
            tc.tile_pool(name="ps", bufs=8, space="PSUM") as psp,
            tc.tile_pool(name="stats", bufs=1) as statsp,
            tc.tile_pool(name="io", bufs=4) as iop,
            tc.tile_pool(name="work", bufs=4) as workp,
            tc.tile_pool(name="attnT", bufs=1) as attnTp,
        ):
            ident = constp.tile([P, P], BF16)
            make_identity(nc, ident[:])
            eps_t = constp.tile([P, 1], F32)
            nc.gpsimd.memset(eps_t[:], EPS)
            mask_sb = constp.tile([P, NB * 2 * P], BF16)
            for j in range(NB):
                for i in range(2):
                    col = (j * 2 + i) * P
                    nc.sync.dma_start(
                        mask_sb[:, col : col + P], maskt[j, i, :, :]
                    )
            mask_v = mask_sb[:].rearrange("p (a b q) -> p a b q", a=NB, b=2)

            # attn^T tiles, pair-major: tile p holds heads 2p (partitions
            # 0:64) and 2p+1 (64:128) over the owned 1024 q columns --
            # exactly the dm-tile layout the O projection needs.
            attnT = [
                attnTp.tile([P, NB * P], BF16, tag=f"aT{j}", name=f"aT{j}")
                for j in range(ND)
            ]
            attn_nat = [
                attnTp.tile([P, D], BF16, tag=f"at{b}", name=f"at{b}")
                for b in range(NB)
            ]

            def transpose_128(dst_ap, src_ap):
                """dst[128, 128] = src[128, 128].T via PE transpose + DVE
                evict.  (DMA X-bar transpose serializes against every other
                DMA through the xbar-mode switch -- measured ~1.3us each.)"""
                pst = psp.tile([P, P], BF16, tag="ps", name="pst")
                nc.tensor.transpose(pst[:], src_ap, ident[:])
                nc.vector.tensor_copy(dst_ap, pst[:])

            with (
                tc.tile_pool(name="xT", bufs=1) as xTp,
                tc.tile_pool(name="xqT", bufs=1) as xqTp,
            ):
                xT = [
                    xTp.tile([P, T], BF16, tag=f"xT{j}", name=f"xT{j}")
                    for j in range(ND)
                ]
                xqT = [
                    xqTp.tile([P, NB * P], BF16, tag=f"xqT{j}", name=f"xqT{j}")
                    for j in range(ND)
                ]

                # ---- Stage A: stats + normalize + transpose (x and xq)
                with tc.tile_pool(name="xnat", bufs=1) as xnatp:
                    sumsq = statsp.tile([P, NT], F32)
                    rms = statsp.tile([P, NT], F32)
                    invr = statsp.tile([P, NT], F32)
                    sumsq_q = statsp.tile([P, NB], F32)
                    rms_q = statsp.tile([P, NB], F32)
                    invr_q = statsp.tile([P, NB], F32)
                    xnat = []
                    xqnat = []
                    for i in range(NT):
                        xt = xnatp.tile([P, D], BF16, tag=f"xn{i}")
                        nc.sync.dma_start(xt[:], xbt[i * P : (i + 1) * P, :])
                        scr = workp.tile([P, D], BF16, tag="sq_scr")
                        nc.scalar.activation(
                            scr[:], xt[:], AF.Square,
                            accum_out=sumsq[:, i : i + 1],
                        )
                        xnat.append(xt)
                    for i in range(NB):
                        xt = xnatp.tile([P, D], BF16, tag=f"xqn{i}")
                        nc.sync.dma_start(xt[:], xq[i * P : (i + 1) * P, :])
                        scr = workp.tile([P, D], BF16, tag="sq_scr")
                        nc.scalar.activation(
                            scr[:], xt[:], AF.Square,
                            accum_out=sumsq_q[:, i : i + 1],
                        )
                        xqnat.append(xt)

                    # per-tile sqrt/recip so normalize+transpose of tile i
                    # doesn't wait for all tiles' stats
                    for i in range(NT):
                        nc.scalar.activation(
                            rms[:, i : i + 1], sumsq[:, i : i + 1],
                            AF.Sqrt, bias=eps_t[:], scale=1.0 / D,
                        )
                        nc.vector.reciprocal(
                            invr[:, i : i + 1], rms[:, i : i + 1]
                        )
                        xn = workp.tile([P, D], BF16, tag="xn")
                        nc.scalar.activation(
                            xn[:], xnat[i][:], AF.Copy,
                            scale=invr[:, i : i + 1],
                        )
                        for j in range(ND):
                            transpose_128(
                                xT[j][:, i * P : (i + 1) * P],
                                xn[:, j * P : (j + 1) * P],
                            )
                    for i in range(NB):
                        nc.scalar.activation(
                            rms_q[:, i : i + 1], sumsq_q[:, i : i + 1],
                            AF.Sqrt, bias=eps_t[:], scale=1.0 / D,
                        )
                        nc.vector.reciprocal(
                            invr_q[:, i : i + 1], rms_q[:, i : i + 1]
                        )
                        xn = workp.tile([P, D], BF16, tag="xn")
                        nc.scalar.activation(
                            xn[:], xqnat[i][:], AF.Copy,
                            scale=invr_q[:, i : i + 1],
                        )
                        for j in range(ND):
                            transpose_128(
                                xqT[j][:, i * P : (i + 1) * P],
                                xn[:, j * P : (j + 1) * P],
                            )

                # ---- Stage B: V (natural layout) with ones column appended
                with tc.tile_pool(name="vaug", bufs=1) as vaugp:
                    vaug = [
                        vaugp.tile(
                            [P, NH * (DH + 1)], BF16,
                            tag=f"va{t}", name=f"va{t}",
                        )
                        for t in range(NT)
                    ]
                    with tc.tile_pool(name="wv", bufs=1) as wvp:
                        wv_t = [
                            wvp.tile([P, D], BF16, tag=f"wv{j}", name=f"wv{j}")
                            for j in range(ND)
                        ]
                        for j in range(ND):
                            nc.sync.dma_start(
                                wv_t[j][:],
                                wqkv[j * P : (j + 1) * P, 2 * D : 3 * D],
                            )
                        for t in range(NT):
                            ones_view = vaug[t][:].rearrange(
                                "p (h c) -> p h c", c=DH + 1
                            )[:, :, DH : DH + 1]
                            nc.gpsimd.memset(ones_view, 1.0)
                            psv = [
                                psp.tile(
                                    [P, 512], F32, tag="ps", name=f"psv{cs}"
                                )
                                for cs in range(2)
                            ]
                            # j outer so both c-spans share one LDWEIGHTS
                            for j in range(ND):
                                for cs in range(2):
                                    nc.tensor.matmul(
                                        psv[cs][:],
                                        lhsT=xT[j][:, t * P : (t + 1) * P],
                                        rhs=wv_t[j][
                                            :, cs * 512 : (cs + 1) * 512
                                        ],
                                        start=(j == 0),
                                        stop=(j == ND - 1),
                                    )
                            for cs in range(2):
                                dst = vaug[t][:].rearrange(
                                    "p (h c) -> p h c", c=DH + 1
                                )[:, cs * 8 : (cs + 1) * 8, 0:DH]
                                src = psv[cs][:].rearrange(
                                    "p (a c) -> p a c", c=DH
                                )
                                nc.vector.tensor_copy(dst, src)

                    # ---- Stage C: per head pair: K^T/Q^T proj + attention
                    with (
                        tc.tile_pool(name="wqk", bufs=1) as wqkp,
                        tc.tile_pool(name="ktqt", bufs=2) as ktqtp,
                        tc.tile_pool(name="pt", bufs=8) as ptp,
                    ):
                        wq_t = [
                            wqkp.tile([P, D], BF16, tag=f"wq{j}", name=f"wq{j}")
                            for j in range(ND)
                        ]
                        wk_t = [
                            wqkp.tile([P, D], BF16, tag=f"wk{j}", name=f"wk{j}")
                            for j in range(ND)
                        ]
                        for j in range(ND):
                            nc.sync.dma_start(
                                wq_t[j][:], wqkv[j * P : (j + 1) * P, 0:D]
                            )
                            nc.sync.dma_start(
                                wk_t[j][:], wqkv[j * P : (j + 1) * P, D : 2 * D]
                            )

                        for p in range(NH // 2):
                            KT = ktqtp.tile([P, T], BF16, tag="KT")
                            QT = ktqtp.tile([P, NB * P], BF16, tag="QT")
                            # j outer in groups of 2 t-spans: one LDWEIGHTS
                            # per 2 matmuls while holding only 2 psum banks
                            # (4 banks starved the concurrent attention spans)
                            for tg in range(2):
                                psk = [
                                    psp.tile(
                                        [P, 512], F32, tag="ps",
                                        name=f"psk{ts}",
                                    )
                                    for ts in range(2)
                                ]
                                for j in range(ND):
                                    for ts in range(2):
                                        t4 = tg * 2 + ts
                                        nc.tensor.matmul(
                                            psk[ts][:],
                                            lhsT=wk_t[j][
                                                :, p * P : (p + 1) * P
                                            ],
                                            rhs=xT[j][
                                                :, t4 * 512 : (t4 + 1) * 512
                                            ],
                                            start=(j == 0),
                                            stop=(j == ND - 1),
                                        )
                                for ts in range(2):
                                    t4 = tg * 2 + ts
                                    nc.vector.tensor_copy(
                                        KT[:, t4 * 512 : (t4 + 1) * 512],
                                        psk[ts][:],
                                    )
                            psq = [
                                psp.tile(
                                    [P, 512], F32, tag="ps", name=f"psq{ts}"
                                )
                                for ts in range(2)
                            ]
                            for j in range(ND):
                                for ts in range(2):
                                    nc.tensor.matmul(
                                        psq[ts][:],
                                        lhsT=wq_t[j][:, p * P : (p + 1) * P],
                                        rhs=xqT[j][
                                            :, ts * 512 : (ts + 1) * 512
                                        ],
                                        start=(j == 0),
                                        stop=(j == ND - 1),
                                    )
                            for ts in range(2):
                                nc.vector.tensor_copy(
                                    QT[:, ts * 512 : (ts + 1) * 512],
                                    psq[ts][:],
                                )

                            # attention over q-spans of 256 (owned blocks
                            # 2s, 2s+1); k extent padded to 4s+4 tiles so
                            # the loop structure is h-independent.
                            for s in range(4):
                                nk = 4 * s + 4
                                psav = {}
                                for bl in range(2):
                                    for hl in range(2):
                                        psav[(bl, hl)] = psp.tile(
                                            [P, DH + 1], F32, tag="ps",
                                            name=f"psav{bl}{hl}",
                                        )
                                # two k-tiles share one psum bank + one exp
                                for kt2 in range(nk // 2):
                                    for hl in range(2):
                                        pss = psp.tile([P, 512], F32, tag="ps")
                                        for ktl in range(2):
                                            kt = 2 * kt2 + ktl
                                            nc.tensor.matmul(
                                                pss[
                                                    :,
                                                    ktl * 256 : (ktl + 1) * 256,
                                                ],
                                                lhsT=KT[
                                                    hl * DH : (hl + 1) * DH,
                                                    kt * P : (kt + 1) * P,
                                                ],
                                                rhs=QT[
                                                    hl * DH : (hl + 1) * DH,
                                                    s * 256 : (s + 1) * 256,
                                                ],
                                                start=True,
                                                stop=True,
                                            )
                                        pt = ptp.tile([P, 512], BF16, tag="pt")
                                        nc.scalar.activation(
                                            pt[:], pss[:], AF.Exp, scale=0.125
                                        )
                                        hh = 2 * p + hl
                                        for ktl in range(2):
                                            kt = 2 * kt2 + ktl
                                            ko = ktl * 256
                                            # diagonal-block masks
                                            for bl in range(2):
                                                b = 2 * s + bl
                                                if kt in (2 * b, 2 * b + 1):
                                                    i = kt - 2 * b
                                                    sl = pt[
                                                        :,
                                                        ko + bl * P : ko
                                                        + (bl + 1) * P,
                                                    ]
                                                    nc.vector.tensor_mul(
                                                        sl,
                                                        sl,
                                                        mask_v[:, b, i, :],
                                                    )
                                            # natural-layout AV + rowsum:
                                            # out[q, 0:64] = P @ V_head,
                                            # out[q, 64] = softmax denom
                                            for bl in range(2):
                                                b = 2 * s + bl
                                                lb = 2 * b + 2
                                                if kt < lb:
                                                    nc.tensor.matmul(
                                                        psav[(bl, hl)][:],
                                                        lhsT=pt[
                                                            :,
                                                            ko
                                                            + bl * P : ko
                                                            + (bl + 1) * P,
                                                        ],
                                                        rhs=vaug[kt][
                                                            :,
                                                            hh
                                                            * (DH + 1) : (hh + 1)
                                                            * (DH + 1),
                                                        ],
                                                        start=(kt == 0),
                                                        stop=(kt == lb - 1),
                                                    )
                                for bl in range(2):
                                    for hl in range(2):
                                        b = 2 * s + bl
                                        hh = 2 * p + hl
                                        rec = workp.tile(
                                            [P, 1], F32, tag="rec"
                                        )
                                        nc.vector.reciprocal(
                                            rec[:],
                                            psav[(bl, hl)][:, DH : DH + 1],
                                        )
                                        nc.vector.tensor_scalar_mul(
                                            attn_nat[b][
                                                :, hh * DH : (hh + 1) * DH
                                            ],
                                            psav[(bl, hl)][:, 0:DH],
                                            rec[:],
                                        )

            # ---- Stage D: transpose attn to attnT (pair-major dm tiles)
            for b in range(NB):
                for j in range(ND):
                    transpose_128(
                        attnT[j][:, b * P : (b + 1) * P],
                        attn_nat[b][:, j * P : (j + 1) * P],
                    )

            # ---- Stage E: O proj + residual, rmsnorm2
            with tc.tile_pool(name="h2", bufs=1) as h2p:
                h2 = [
                    h2p.tile([P, D], F32, tag=f"h2{b}", name=f"h2{b}")
                    for b in range(NB)
                ]
                with (
                    tc.tile_pool(name="wo", bufs=1) as wop,
                    tc.tile_pool(name="io2", bufs=4) as io2p,
                ):
                    wo_t = [
                        wop.tile([P, D], BF16, tag=f"wo{j}", name=f"wot{j}")
                        for j in range(ND)
                    ]
                    for j in range(ND):
                        nc.sync.dma_start(
                            wo_t[j][:], wo[j * P : (j + 1) * P, :]
                        )
                    sumsq2 = statsp.tile([P, NB], F32)
                    for b in range(NB):
                        xr = io2p.tile([P, D], F32, tag="xres")
                        nc.sync.dma_start(
                            xr[:], xres[b * P : (b + 1) * P, :]
                        )
                        pso = [
                            psp.tile([P, 512], F32, tag="ps", name=f"pso{es}")
                            for es in range(2)
                        ]
                        # j outer: both e-spans share one LDWEIGHTS
                        for j in range(ND):
                            for es in range(2):
                                nc.tensor.matmul(
                                    pso[es][:],
                                    lhsT=attnT[j][:, b * P : (b + 1) * P],
                                    rhs=wo_t[j][:, es * 512 : (es + 1) * 512],
                                    start=(j == 0),
                                    stop=(j == ND - 1),
                                )
                        for es in range(2):
                            nc.vector.tensor_add(
                                h2[b][:, es * 512 : (es + 1) * 512],
                                pso[es][:],
                                xr[:, es * 512 : (es + 1) * 512],
                            )
                        scr = workp.tile([P, D], BF16, tag="sq_scr")
                        nc.scalar.activation(
                            scr[:], h2[b][:], AF.Square,
                            accum_out=sumsq2[:, b : b + 1],
                        )

                # ---- Stage F: MLP (transposed up/gate, natural down)
                with (
                    tc.tile_pool(name="hnT", bufs=1) as hnTp,
                    tc.tile_pool(name="mlpT", bufs=1) as mlpTp,
                    tc.tile_pool(name="ws", bufs=3) as wsp,
                    tc.tile_pool(name="io3", bufs=4) as io3p,
                ):
                    rms2 = statsp.tile([P, NB], F32)
                    invr2 = statsp.tile([P, NB], F32)

                    hnT = [
                        hnTp.tile(
                            [P, NB * P], BF16, tag=f"hT{j}", name=f"hT{j}"
                        )
                        for j in range(ND)
                    ]
                    for b in range(NB):
                        nc.scalar.activation(
                            rms2[:, b : b + 1], sumsq2[:, b : b + 1],
                            AF.Sqrt, bias=eps_t[:], scale=1.0 / D,
                        )
                        nc.vector.reciprocal(
                            invr2[:, b : b + 1], rms2[:, b : b + 1]
                        )
                        hn = workp.tile([P, D], BF16, tag="xn")
                        nc.scalar.activation(
                            hn[:], h2[b][:], AF.Copy,
                            scale=invr2[:, b : b + 1],
                        )
                        for j in range(ND):
                            transpose_128(
                                hnT[j][:, b * P : (b + 1) * P],
                                hn[:, j * P : (j + 1) * P],
                            )

                    mlpT = [
                        mlpTp.tile(
                            [P, NB * P], BF16, tag=f"m{ft}", name=f"mT{ft}"
                        )
                        for ft in range(NF)
                    ]
                    for ft in range(NF):
                        wu = wsp.tile([P, D], BF16, tag="wu")
                        nc.sync.dma_start(
                            wu[:].rearrange("p (a f) -> p a f", f=P),
                            wup[:, ft * P : (ft + 1) * P].rearrange(
                                "(a p) f -> p a f", p=P
                            ),
                        )
                        wg = wsp.tile([P, D], BF16, tag="wg")
                        nc.sync.dma_start(
                            wg[:].rearrange("p (a f) -> p a f", f=P),
                            wgate[:, ft * P : (ft + 1) * P].rearrange(
                                "(a p) f -> p a f", p=P
                            ),
                        )
                        psg = [
                            psp.tile([P, 512], F32, tag="ps", name=f"psg{qs}")
                            for qs in range(2)
                        ]
                        psu = [
                            psp.tile([P, 512], F32, tag="ps", name=f"psu{qs}")
                            for qs in range(2)
                        ]
                        # j outer: both q-spans share one LDWEIGHTS
                        for j in range(ND):
                            for qs in range(2):
                                nc.tensor.matmul(
                                    psg[qs][:],
                                    lhsT=wg[:, j * P : (j + 1) * P],
                                    rhs=hnT[j][:, qs * 512 : (qs + 1) * 512],
                                    start=(j == 0),
                                    stop=(j == ND - 1),
                                )
                        for j in range(ND):
                            for qs in range(2):
                                nc.tensor.matmul(
                                    psu[qs][:],
                                    lhsT=wu[:, j * P : (j + 1) * P],
                                    rhs=hnT[j][:, qs * 512 : (qs + 1) * 512],
                                    start=(j == 0),
                                    stop=(j == ND - 1),
                                )
                        for qs in range(2):
                            if sim_compat:
                                # CoreSim lacks Silu: silu(g) = g*sigmoid(g)
                                sg = workp.tile([P, 512], BF16, tag="sg")
                                nc.scalar.activation(
                                    sg[:], psg[qs][:], AF.Sigmoid
                                )
                                tmp = workp.tile([P, 512], BF16, tag="sgt")
                                nc.vector.tensor_mul(
                                    tmp[:], psg[qs][:], sg[:]
                                )
                                nc.vector.tensor_mul(
                                    mlpT[ft][:, qs * 512 : (qs + 1) * 512],
                                    psu[qs][:],
                                    tmp[:],
                                )
                            else:
                                sg = workp.tile([P, 512], BF16, tag="sg")
                                nc.scalar.activation(
                                    sg[:], psg[qs][:], AF.Silu
                                )
                                nc.vector.tensor_mul(
                                    mlpT[ft][:, qs * 512 : (qs + 1) * 512],
                                    psu[qs][:],
                                    sg[:],
                                )

                    # down projection + final residual, four q-blocks at a
                    # time; each mlpT LDWEIGHTS serves both e-spans
                    for bh in range(2):
                        psd = {}
                        for bi in range(4):
                            for es in range(2):
                                psd[(bi, es)] = psp.tile(
                                    [P, 512], F32, tag="ps",
                                    name=f"psd{bi}{es}",
                                )
                        for ft in range(NF):
                            wd = wsp.tile([P, D], BF16, tag="wd")
                            nc.sync.dma_start(
                                wd[:], wdown[ft * P : (ft + 1) * P, :]
                            )
                            for bi in range(4):
                                b = bh * 4 + bi
                                for es in range(2):
                                    nc.tensor.matmul(
                                        psd[(bi, es)][:],
                                        lhsT=mlpT[ft][:, b * P : (b + 1) * P],
                                        rhs=wd[:, es * 512 : (es + 1) * 512],
                                        start=(ft == 0),
                                        stop=(ft == NF - 1),
                                    )
                        for bi in range(4):
                            b = bh * 4 + bi
                            for es in range(2):
                                ot = io3p.tile([P, 512], F32, tag="outt")
                                nc.vector.tensor_add(
                                    ot[:],
                                    psd[(bi, es)][:],
                                    h2[b][:, es * 512 : (es + 1) * 512],
                                )
                                nc.sync.dma_start(
                                    out[
                                        b * P : (b + 1) * P,
                                        es * 512 : (es + 1) * 512,
                                    ],
                                    ot[:],
                                )

    orig_to_json_bytes = nc.to_json_bytes

    def _patched_to_json_bytes():
        bir = orjson.loads(orig_to_json_bytes())
        bir = _split_multi_waits(bir)
        bir = _dedupe_ldweights(bir)
        return orjson.dumps(bir)

    nc.to_json_bytes = _patched_to_json_bytes
    return nc


_NC_CACHE = {}


def _get_nc(sim_compat=False):
    if sim_compat not in _NC_CACHE:
        _NC_CACHE[sim_compat] = build_nc(sim_compat)
    return _NC_CACHE[sim_compat]


def _prep_core_inputs(x, w_qkv, w_o, w_up, w_gate, w_down, scale1, scale2):
    bf = ml_dtypes.bfloat16
    wqkv_f = (scale1[:, None].astype(np.float64) * w_qkv.astype(np.float64))
    wup_f = (scale2[:, None].astype(np.float64) * w_up.astype(np.float64))
    wgate_f = (scale2[:, None].astype(np.float64) * w_gate.astype(np.float64))
    shared = {
        "wqkv": wqkv_f.astype(bf),
        "wo": w_o.astype(bf),
        "wup": wup_f.astype(bf),
        "wgate": wgate_f.astype(bf),
        "wdown": w_down.astype(bf),
    }
    in_maps = []
    for c in range(N_CORES):
        b, h = divmod(c, 2)
        xb = np.asarray(x[b], dtype=np.float32)
        own = np.concatenate(
            [xb[(2 * j + h) * P : (2 * j + h + 1) * P] for j in range(NB)]
        )
        mask = np.zeros((NB, 2, P, P), dtype=np.float32)
        kl = np.arange(P)[:, None]
        ql = np.arange(P)[None, :]
        for j in range(NB):
            g = 2 * j + h
            for i in range(2):
                kg = (2 * j + i) * P + kl
                qg = g * P + ql
                mask[j, i] = (kg <= qg).astype(np.float32)
        m = dict(shared)
        m["xbt"] = xb.astype(bf)
        m["xq"] = own.astype(bf)
        m["xres"] = own
        m["maskt"] = mask.astype(bf)
        in_maps.append(m)
    return in_maps


def _assemble(results):
    out = np.zeros((4, T, D), dtype=np.float32)
    for c in range(N_CORES):
        b, h = divmod(c, 2)
        o = results[c]["out"]
        for j in range(NB):
            g = 2 * j + h
            out[b, g * P : (g + 1) * P, :] = o[j * P : (j + 1) * P, :]
    return out


def kernel(x, w_qkv, w_o, w_up, w_gate, w_down, scale1, scale2):
    x = np.asarray(x, dtype=np.float32)
    in_maps = _prep_core_inputs(
        x,
        np.asarray(w_qkv, dtype=np.float32),
        np.asarray(w_o, dtype=np.float32),
        np.asarray(w_up, dtype=np.float32),
        np.asarray(w_gate, dtype=np.float32),
        np.asarray(w_down, dtype=np.float32),
        np.asarray(scale1, dtype=np.float32),
        np.asarray(scale2, dtype=np.float32),
    )
    nc = _get_nc()
    res = run_bass_kernel_spmd(nc, in_maps, list(range(N_CORES)))
    return _assemble(res.results)
